# revision 4
# baseline (speedup 1.0000x reference)
"""Trainium2 Bass kernel for masked cross-attention (nn_CausalAttention).

Reference computation (per batch):
    q  = x @ Wq                       # [128, 1024]
    kv = context @ Wkv; k, v = split  # [4096, 1024] each
    per head h (16 heads, dim 64):
        sim[i, j] = (q_h[i] . k_h[j]) * 0.125, masked to j % 128 == i
        out_h = softmax(sim) @ v_h
    y = concat_h(out) @ Wout + bout

The mask (j % 128) == i means query i attends exactly the 32 keys
j = i + 128*t.  KV-projection token-tile t lands in SBUF as
[128 tokens, 1024 feats] with token i on partition i, so the scores are
per-partition dot products (DVE elementwise mul + segmented reduce) and the
attention-weighted V sum is a per-partition broadcast-mul accumulate.  The
dense [128, 4096] similarity matrix is never formed.

Sharding: data-parallel over batch, 2 batches per core, no collectives.
Host pre-transposes x and context to feat-major so every matmul operand has
the contraction dim on partitions with no on-chip transposes.  Matmuls run
in float32r (single-pass fp22).
"""

import numpy as np
from contextlib import ExitStack

import concourse.bass as bass
import concourse.tile as tile
from concourse import bacc, mybir
from concourse.bass_utils import run_bass_kernel_spmd
from concourse.masks import make_identity

FP = mybir.dt.float32
FPR = mybir.dt.float32r
AX = mybir.AxisListType
ALU = mybir.AluOpType
ACTF = mybir.ActivationFunctionType

B, NQ, NKV, DIM, H, DH = 16, 128, 4096, 1024, 16, 64
INNER = H * DH  # 1024
SCALE = DH ** -0.5  # 0.125
N_CORES = 8
BPC = B // N_CORES  # batches per core
KT = DIM // 128     # 8 contraction chunks
NT = INNER // 512   # 2 output-feature chunks of 512
TT = NKV // NQ      # 32 key tiles per query row
TG = 4              # t-tiles per ctx strip load ([128, 512] strips)


def _body(tc, xT, ctxT, wq, wkv, wout, bout, y, bpc=BPC):
    nc = tc.nc
    BPC = bpc
    with ExitStack() as ctx:
        ep = ctx.enter_context

        wkv_p = ep(tc.tile_pool(name="wkv", bufs=2 * KT * NT))      # 64KB/part
        wqo_p = ep(tc.tile_pool(name="wqo", bufs=KT * NT))          # 32KB/part
        ctxs_p = ep(tc.tile_pool(name="ctxs", bufs=10))             # 20KB/part
        xt_p = ep(tc.tile_pool(name="xt", bufs=KT))
        q_p = ep(tc.tile_pool(name="q", bufs=BPC))
        kvt_p = ep(tc.tile_pool(name="kvt", bufs=2))
        prod_p = ep(tc.tile_pool(name="prod", bufs=2))
        acc_p = ep(tc.tile_pool(name="acc", bufs=2))
        sim_p = ep(tc.tile_pool(name="sim", bufs=2))
        exp_p = ep(tc.tile_pool(name="exp", bufs=2))
        stat_p = ep(tc.tile_pool(name="stat", bufs=8))
        ot_p = ep(tc.tile_pool(name="ot", bufs=KT))
        yb_p = ep(tc.tile_pool(name="yb", bufs=1))
        const_p = ep(tc.tile_pool(name="const", bufs=2))
        psum_p = ep(tc.tile_pool(name="psum", bufs=4, space="PSUM"))
        psum_tr_p = ep(tc.tile_pool(name="psumtr", bufs=2, space="PSUM"))

        ident = const_p.tile([128, 128], FP, tag="ident")
        make_identity(nc, ident[:])
        bout_sb = const_p.tile([128, INNER], FP, tag="bout")
        nc.sync.dma_start(bout_sb[:], bout[:, :])

        # ---- weights ----
        wq_t = {}
        for k in range(KT):
            for n in range(NT):
                t = wqo_p.tile([128, 512], FPR, tag="wqo")
                nc.sync.dma_start(
                    t[:], wq[k * 128:(k + 1) * 128,
                             n * 512:(n + 1) * 512].bitcast(FPR))
                wq_t[k, n] = t
        wk_t, wv_t = {}, {}
        for k in range(KT):
            for n in range(NT):
                t = wkv_p.tile([128, 512], FPR, tag="wkv")
                nc.sync.dma_start(
                    t[:], wkv[k * 128:(k + 1) * 128,
                              n * 512:(n + 1) * 512].bitcast(FPR))
                wk_t[k, n] = t
                t = wkv_p.tile([128, 512], FPR, tag="wkv")
                nc.sync.dma_start(
                    t[:], wkv[k * 128:(k + 1) * 128,
                              INNER + n * 512:INNER + (n + 1) * 512].bitcast(FPR))
                wv_t[k, n] = t

        # ---- Q projection (both batches), scores scale folded into evac ----
        q_sb = []
        for b in range(BPC):
            xt = []
            for k in range(KT):
                t = xt_p.tile([128, 128], FPR, tag="xt")
                nc.sync.dma_start(
                    t[:], xT[b, k * 128:(k + 1) * 128, :].bitcast(FPR))
                xt.append(t)
            q = q_p.tile([128, INNER], FP, tag="q")
            for n in range(NT):
                ps = psum_p.tile([128, 512], FP, tag="ps")
                for k in range(KT):
                    nc.tensor.matmul(
                        ps[:], xt[k][:], wq_t[k, n][:],
                        start=(k == 0), stop=(k == KT - 1))
                nc.scalar.activation(
                    q[:, n * 512:(n + 1) * 512], ps[:], ACTF.Copy, scale=SCALE)
            q_sb.append(q)

        # Wout reuses the Wq pool slots once q-projection has consumed them.
        wout_t = {}
        for k in range(KT):
            for n in range(NT):
                t = wqo_p.tile([128, 512], FPR, tag="wqo")
                nc.sync.dma_start(
                    t[:], wout[k * 128:(k + 1) * 128,
                               n * 512:(n + 1) * 512].bitcast(FPR))
                wout_t[k, n] = t

        def kv_tile(b, t_idx, strips, w_t):
            """Project ctx token-tile t through Wk/Wv half -> SBUF [128, 1024]."""
            tj = t_idx % TG
            kv = kvt_p.tile([128, INNER], FP, tag="kvt")
            for n in range(NT):
                ps = psum_p.tile([128, 512], FP, tag="ps")
                for k in range(KT):
                    lhsT = strips[k][:, tj * 128:(tj + 1) * 128]
                    nc.tensor.matmul(
                        ps[:], lhsT, w_t[k, n][:],
                        start=(k == 0), stop=(k == KT - 1))
                nc.scalar.activation(
                    kv[:, n * 512:(n + 1) * 512], ps[:], ACTF.Copy)
            return kv

        def load_strips(b, tg):
            strips = []
            for k in range(KT):
                s = ctxs_p.tile([128, 128 * TG], FPR, tag="ctxs")
                nc.sync.dma_start(
                    s[:], ctxT[b, k * 128:(k + 1) * 128,
                               tg * 128 * TG:(tg + 1) * 128 * TG].bitcast(FPR))
                strips.append(s)
            return strips

        for b in range(BPC):
            # ---- pass 1: K tiles -> sparse scores sim[i, (h, t)] ----
            sim = sim_p.tile([128, H * TT], FP, tag="sim")
            sim3 = sim[:].rearrange("p (h t) -> p h t", h=H)
            for tg in range(TT // TG):
                strips = load_strips(b, tg)
                for tj in range(TG):
                    t_idx = tg * TG + tj
                    kt = kv_tile(b, t_idx, strips, wk_t)
                    pr = prod_p.tile([128, INNER], FP, tag="prod")
                    nc.vector.tensor_tensor(
                        pr[:], q_sb[b][:], kt[:], op=ALU.mult)
                    nc.vector.reduce_sum(
                        sim3[:, :, t_idx:t_idx + 1],
                        pr[:].rearrange("p (h d) -> p h d", h=H), axis=AX.X)

            # ---- softmax over t (per head) ----
            rmax = stat_p.tile([128, H], FP, tag="rmax")
            nc.vector.reduce_max(rmax[:], sim3, axis=AX.X)
            shift = sim_p.tile([128, H * TT], FP, tag="shift")
            nc.vector.tensor_tensor(
                shift[:].rearrange("p (h t) -> p h t", h=H), sim3,
                rmax[:, :, None].broadcast_to([128, H, TT]), op=ALU.subtract)
            ex = exp_p.tile([128, H * TT], FP, tag="exp")
            nc.scalar.activation(ex[:], shift[:], ACTF.Exp)
            ex3 = ex[:].rearrange("p (h t) -> p h t", h=H)
            den = stat_p.tile([128, H], FP, tag="den")
            nc.vector.reduce_sum(den[:], ex3, axis=AX.X)
            rec = stat_p.tile([128, H], FP, tag="rec")
            nc.vector.reciprocal(rec[:], den[:])

            # ---- pass 2: V tiles -> out_acc[i, (h, d)] = sum_t e[i,h,t]*v_t ----
            acc = None
            for tg in range(TT // TG):
                strips = load_strips(b, tg)
                for tj in range(TG):
                    t_idx = tg * TG + tj
                    vt = kv_tile(b, t_idx, strips, wv_t)
                    ebc = ex3[:, :, t_idx:t_idx + 1].broadcast_to([128, H, DH])
                    vt3 = vt[:].rearrange("p (h d) -> p h d", h=H)
                    if acc is None:
                        acc = acc_p.tile([128, INNER], FP, tag="acc")
                        nc.vector.tensor_tensor(
                            acc[:].rearrange("p (h d) -> p h d", h=H),
                            vt3, ebc, op=ALU.mult)
                    else:
                        wv = prod_p.tile([128, INNER], FP, tag="prod")
                        nc.vector.tensor_tensor(
                            wv[:].rearrange("p (h d) -> p h d", h=H),
                            vt3, ebc, op=ALU.mult)
                        acc2 = acc_p.tile([128, INNER], FP, tag="acc")
                        nc.vector.tensor_tensor(
                            acc2[:], acc[:], wv[:], op=ALU.add)
                        acc = acc2

            # normalize by softmax denominator
            out_n = prod_p.tile([128, INNER], FP, tag="prod")
            nc.vector.tensor_tensor(
                out_n[:].rearrange("p (h d) -> p h d", h=H),
                acc[:].rearrange("p (h d) -> p h d", h=H),
                rec[:, :, None].broadcast_to([128, H, DH]), op=ALU.mult)

            # ---- output projection: transpose out_n, then @ Wout + bout ----
            ot = []
            for k in range(KT):
                pst = psum_tr_p.tile([128, 128], FP, tag="pst")
                nc.tensor.transpose(
                    pst[:], out_n[:, k * 128:(k + 1) * 128], ident[:])
                o = ot_p.tile([128, 128], FPR, tag="ot")
                nc.scalar.activation(o[:], pst[:], ACTF.Copy)
                ot.append(o)
            yb = yb_p.tile([128, INNER], FP, tag="yb")
            for n in range(NT):
                ps = psum_p.tile([128, 512], FP, tag="ps")
                for k in range(KT):
                    nc.tensor.matmul(
                        ps[:], ot[k][:], wout_t[k, n][:],
                        start=(k == 0), stop=(k == KT - 1))
                nc.vector.tensor_tensor(
                    yb[:, n * 512:(n + 1) * 512], ps[:],
                    bout_sb[:, n * 512:(n + 1) * 512], op=ALU.add)
            nc.sync.dma_start(y[b], yb[:])


def build_kernel(bpc=BPC):
    nc = bacc.Bacc("TRN2", target_bir_lowering=False, debug=False)
    xT = nc.dram_tensor("xT", [bpc, DIM, NQ], FP, kind="ExternalInput").ap()
    ctxT = nc.dram_tensor("ctxT", [bpc, DIM, NKV], FP, kind="ExternalInput").ap()
    wq = nc.dram_tensor("wq", [DIM, INNER], FP, kind="ExternalInput").ap()
    wkv = nc.dram_tensor("wkv", [DIM, 2 * INNER], FP, kind="ExternalInput").ap()
    wout = nc.dram_tensor("wout", [INNER, DIM], FP, kind="ExternalInput").ap()
    bout = nc.dram_tensor("bout", [128, DIM], FP, kind="ExternalInput").ap()
    y = nc.dram_tensor("y", [bpc, NQ, DIM], FP, kind="ExternalOutput").ap()

    with tile.TileContext(nc) as tc:
        _body(tc, xT, ctxT, wq, wkv, wout, bout, y, bpc=bpc)
    nc.compile()
    return nc


_NC_CACHE = {}


def make_in_maps(x, context, Wq, Wkv, Wout, bout):
    x = np.ascontiguousarray(x, dtype=np.float32)
    context = np.ascontiguousarray(context, dtype=np.float32)
    bout_rep = np.ascontiguousarray(
        np.broadcast_to(bout.astype(np.float32), (128, DIM)))
    w = {
        "wq": np.ascontiguousarray(Wq, dtype=np.float32),
        "wkv": np.ascontiguousarray(Wkv, dtype=np.float32),
        "wout": np.ascontiguousarray(Wout, dtype=np.float32),
        "bout": bout_rep,
    }
    in_maps = []
    for c in range(N_CORES):
        sl = slice(c * BPC, (c + 1) * BPC)
        xT = np.ascontiguousarray(x[sl].transpose(0, 2, 1))
        ctxT = np.ascontiguousarray(context[sl].transpose(0, 2, 1))
        in_maps.append({"xT": xT, "ctxT": ctxT, **w})
    return in_maps


def kernel(x, context, Wq, Wkv, Wout, bout):
    if "nc" not in _NC_CACHE:
        _NC_CACHE["nc"] = build_kernel()
    nc = _NC_CACHE["nc"]
    in_maps = make_in_maps(x, context, Wq, Wkv, Wout, bout)
    res = run_bass_kernel_spmd(nc, in_maps, list(range(N_CORES)))
    out = np.concatenate([res.results[c]["y"] for c in range(N_CORES)], axis=0)
    return out.astype(np.float32)
